# revision 31
# baseline (speedup 1.0000x reference)
"""SMPL body-model (LBS) kernel for 8 Trainium2 NeuronCores.

Sharding: vertices split across the 8 cores (V=6890 -> 896/core padded);
batch (B=512) replicated on every core.  The host does the tiny per-batch
prep (Rodrigues rotations, the 24-joint forward-kinematics chain, input
layout); the cores run the heavy vertex math:

  vp[b,v,c]    = v_template + shapedirs@betas + posedirs@posefeat
                 (one K=218 matmul per vertex chunk)
  verts[b,v,m] = sum_j w[v,j]*A_t[b,j,m] + trans[b,m]
               + sum_c (sum_j w[v,j]*A_R[b,j,m,c]) * vp[b,v,c]

The 12 skinning contractions per 128-vertex chunk (9 S(m,c) K=24 and
3 At K=25) are packed into distinct 32-row PE tile positions; the
(v,b)-elementwise products and sums are spread across DVE / ACT / GpSimd.
Output is bf16, upcast on the host.
"""
import sys

sys.path.insert(0, '/opt/trn_rl_repo')

import contextlib

import ml_dtypes
import numpy as np

import concourse.bass as bass
import concourse.mybir as mybir
import concourse.tile as tile
from concourse import bacc
from concourse.bass_utils import run_bass_kernel_spmd
from concourse.masks import make_identity

P = 128
B = 512
J = 24
NB = 10
V = 6890
NCORES = 8
VL = 896             # vertices per core (padded)
VC = VL // P         # 7 vertex chunks per core
NPF = 207            # pose-feature length
KD = NPF + NB + 1    # 218 = vp contraction dim; rows [pf(207); betas(10); 1]
KB = KD - P          # 90 = second K chunk

SMPL_PARENTS = np.array([-1, 0, 0, 0, 1, 2, 3, 4, 5, 6, 7, 8, 9, 9, 9, 12,
                         13, 14, 16, 17, 18, 19, 20, 21], dtype=np.int32)

F32 = mybir.dt.float32
BF16 = mybir.dt.bfloat16
MUL = mybir.AluOpType.mult
ADD = mybir.AluOpType.add

# TR4 wave layout (column-batch b_, 32-row group r_):
#   W1 (b_=0): [At0, At1, At2, S22]   At rows get +trans at row 24
#   W2 (b_=1): [S00, S01, S02, S10]
#   W3 (b_=2): [S11, S12, S20, S21]
SLOTS = [('t', 0), ('t', 1), ('t', 2), ('s', 2, 2),
         ('s', 0, 0), ('s', 0, 1), ('s', 0, 2), ('s', 1, 0),
         ('s', 1, 1), ('s', 1, 2), ('s', 2, 0), ('s', 2, 1)]
# products in DRAIN_MC are ACT-drained to bf16 first (cheap DVE multiply);
# the rest multiply straight from PSUM on DVE.  GpSimd is kept out of the
# main loop (it contends with DVE for the shared SBUF port).
DRAIN_MC = ((0, 0), (0, 1), (1, 0), (1, 1), (2, 0))

LAST_RESULTS = None  # for the local test harness


def build_kernel():
    nc = bacc.Bacc("TRN2", target_bir_lowering=False, debug=False,
                   num_devices=NCORES)

    d_bigA = nc.dram_tensor("bigA", [P, 3 * VL], BF16, kind="ExternalInput")
    d_bigB = nc.dram_tensor("bigB", [KB, 3 * VL], BF16, kind="ExternalInput")
    d_wtr4 = nc.dram_tensor("wtr4", [P, VL], BF16, kind="ExternalInput")
    d_phiA = nc.dram_tensor("phiA", [P, B], BF16, kind="ExternalInput")
    d_phiB = nc.dram_tensor("phiB", [KB, B], BF16, kind="ExternalInput")
    d_tr4 = nc.dram_tensor("tr4", [P, 3 * B], BF16, kind="ExternalInput")
    d_out = nc.dram_tensor("out_v", [VL, 3 * B], BF16, kind="ExternalOutput")

    with tile.TileContext(nc) as tc, contextlib.ExitStack() as ctx:
        singles = ctx.enter_context(tc.tile_pool(name="singles", bufs=1))

        # load order: blendshape operands first (they gate the first matmul),
        # skinning operands (wtr4/TR4) later — they are needed only once the
        # first vp tile exists.  bigA/bigB are column-chunked so the vc=0/1
        # slices land first.
        phiA = singles.tile([P, B], BF16)
        nc.sync.dma_start(phiA[:], d_phiA[:, :])
        phiB = singles.tile([KB, B], BF16)
        nc.sync.dma_start(phiB[:], d_phiB[:, :])
        bigA_sb = singles.tile([P, 3 * VL], BF16)
        bigB_sb = singles.tile([KB, 3 * VL], BF16)
        TR4 = singles.tile([P, 3, B], BF16)
        wtr4_sb = singles.tile([P, VL], BF16)
        for half in range(2):
            lo = [0, 2 * P][half]
            hi = [2 * P, VL][half]
            for c in range(3):
                nc.sync.dma_start(bigA_sb[:, c * VL + lo:c * VL + hi],
                                  d_bigA[:, c * VL + lo:c * VL + hi])
                nc.scalar.dma_start(bigB_sb[:, c * VL + lo:c * VL + hi],
                                    d_bigB[:, c * VL + lo:c * VL + hi])
            if half == 0:
                nc.scalar.dma_start(wtr4_sb[:], d_wtr4[:, :])
                nc.sync.dma_start(
                    TR4[:], d_tr4[:, :].rearrange("p (t b) -> p t b", t=3))
        ident_bf = singles.tile([P, P], BF16)
        make_identity(nc, ident_bf[:])

        dvp_pool = ctx.enter_context(tc.tile_pool(name="dvp", bufs=3))
        tmp_pool = ctx.enter_context(tc.tile_pool(name="tmpmc", bufs=2))
        out_pool = ctx.enter_context(tc.tile_pool(name="outs", bufs=2))
        ps_d = ctx.enter_context(tc.tile_pool(name="psD", bufs=2, space="PSUM"))
        ps_s = ctx.enter_context(tc.tile_pool(name="psS", bufs=2, space="PSUM"))
        ps_t = ctx.enter_context(tc.tile_pool(name="psT", bufs=1, space="PSUM"))

        # PE warm-up: a gapless burst of throwaway matmuls during the
        # input-DMA head flips the HAM clock gate to 8/8 (2.4 GHz) before
        # the real matmul stream begins; main-loop gaps are well under the
        # ~3.4us re-throttle window, so the PE stays warm afterwards.
        warm = ps_d.tile([P, B], F32, tag="dvpp")
        for _ in range(36):
            nc.tensor.matmul(warm[:, 0:P], ident_bf[:], ident_bf[:],
                             start=True, stop=True)

        def emit_dvp(vc):
            dvp_sb = dvp_pool.tile([P, 3, B], BF16, tag="dvp")
            for c in range(3):
                dps = ps_d.tile([P, B], F32, tag="dvpp")
                nc.tensor.matmul(
                    dps[:],
                    bigA_sb[:, c * VL + vc * P: c * VL + (vc + 1) * P],
                    phiA[:], start=True, stop=False)
                nc.tensor.matmul(
                    dps[:],
                    bigB_sb[:, c * VL + vc * P: c * VL + (vc + 1) * P],
                    phiB[:], start=False, stop=True)
                nc.scalar.copy(dvp_sb[:, c, :], dps[:])
            return dvp_sb

        dvp_tiles = {}
        dvp_tiles[0] = emit_dvp(0)
        dvp_tiles[1] = emit_dvp(1)

        # second warm-up burst: bridges the PE-idle bubble while the
        # skinning operands (TR4/wtr4) finish loading, so the HAM gate
        # stays at 8/8 into the main loop
        warm2 = ps_d.tile([P, B], F32, tag="dvpp")
        for _ in range(30):
            nc.tensor.matmul(warm2[:, 0:P], ident_bf[:], ident_bf[:],
                             start=True, stop=True)

        for vc in range(VC):
            vsl = slice(vc * P, (vc + 1) * P)
            dvp_sb = dvp_tiles.pop(vc)
            s_ps = {}
            t_ps = {}
            for q, slot in enumerate(SLOTS):
                b_, r_ = q // 4, q % 4
                if slot[0] == 't':
                    m = slot[1]
                    tp = ps_t.tile([P, B], F32, tag=f"tps{m}", bufs=1)
                    nc.tensor.matmul(tp[:],
                                     wtr4_sb[32 * r_:32 * r_ + J + 1, vsl],
                                     TR4[32 * r_:32 * r_ + J + 1, b_, :],
                                     start=True, stop=True,
                                     tile_position=(32 * r_, 0))
                    t_ps[m] = tp
                else:
                    m, c = slot[1], slot[2]
                    if (m, c) == (2, 2):
                        sp = ps_s.tile([P, B], F32, tag="s22", bufs=1)
                    else:
                        sp = ps_s.tile([P, B], F32, tag="sps")
                    nc.tensor.matmul(sp[:], wtr4_sb[32 * r_:32 * r_ + J, vsl],
                                     TR4[32 * r_:32 * r_ + J, b_, :],
                                     start=True, stop=True,
                                     tile_position=(32 * r_, 0))
                    s_ps[(m, c)] = sp
            # prefetch the vp tile two chunks ahead
            if vc + 2 < VC:
                dvp_tiles[vc + 2] = emit_dvp(vc + 2)
            # drain the At banks (needed in SBUF for the final merges)
            tt_sb = {}
            for m in range(3):
                tb = tmp_pool.tile([P, B], BF16, tag=f"tt{m}")
                nc.scalar.copy(tb[:], t_ps[m][:])
                tt_sb[m] = tb
            # products P(m,c) = S(m,c) * vp[c], straight from PSUM on DVE
            prod = {}
            for mc in [(0, 0), (0, 1), (0, 2), (1, 0), (1, 1), (1, 2),
                       (2, 0), (2, 1), (2, 2)]:
                m, c = mc
                pm = tmp_pool.tile([P, B], BF16, tag=f"pr{m}{c}")
                if mc in DRAIN_MC:
                    sb = tmp_pool.tile([P, B], BF16, tag=f"sdr{m}{c}")
                    nc.scalar.copy(sb[:], s_ps[mc][:])
                    nc.vector.tensor_tensor(pm[:], sb[:],
                                            dvp_sb[:, c, :], MUL)
                else:
                    nc.vector.tensor_tensor(pm[:], s_ps[mc][:],
                                            dvp_sb[:, c, :], MUL)
                prod[mc] = pm
            # shallow add tree per m: (P0+P1) + (P2+At) on DVE
            for m in range(3):
                t01 = tmp_pool.tile([P, B], BF16, tag=f"t01_{m}")
                nc.vector.tensor_tensor(t01[:], prod[(m, 0)][:],
                                        prod[(m, 1)][:], ADD)
                t2t = tmp_pool.tile([P, B], BF16, tag=f"t2t_{m}")
                nc.vector.tensor_tensor(t2t[:], prod[(m, 2)][:],
                                        tt_sb[m][:], ADD)
                vo = tmp_pool.tile([P, B], BF16, tag=f"vo_{m}")
                nc.vector.tensor_tensor(vo[:], t01[:], t2t[:], ADD)
                nc.sync.dma_start(d_out[vsl, m * B:(m + 1) * B], vo[:])

    nc.compile()
    return nc


_NC_CACHE = None


def _get_nc():
    global _NC_CACHE
    if _NC_CACHE is None:
        _NC_CACHE = build_kernel()
    return _NC_CACHE


def _host_prep(pose, betas, trans, v_template, shapedirs, J_regressor):
    """Per-batch prep: Rodrigues + FK chain -> A matrices, pose features."""
    rvec = pose.reshape(B, J, 3).astype(np.float64)
    angle = np.sqrt((rvec * rvec).sum(-1, keepdims=True) + 1e-16)
    axis = rvec / angle
    s = np.sin(angle)[..., None]
    c = np.cos(angle)[..., None]
    x, y, z = axis[..., 0], axis[..., 1], axis[..., 2]
    zero = np.zeros_like(x)
    K = np.stack([zero, -z, y, z, zero, -x, -y, x, zero],
                 axis=-1).reshape(B, J, 3, 3)
    outer = axis[..., :, None] * axis[..., None, :]
    rot = c * np.eye(3)[None, None] + (1.0 - c) * outer + s * K  # [B,J,3,3]

    pf = (rot[:, 1:] - np.eye(3)[None, None]).reshape(B, NPF)    # [B,207]

    # joint rest positions from shape blendshapes
    j0 = J_regressor @ v_template                                 # [24,3]
    jd = (J_regressor @ shapedirs.reshape(V, 30)).reshape(J, 3, NB)
    Jts = j0[None] + np.einsum('bk,jck->bjc', betas.astype(np.float64),
                               jd.astype(np.float64))             # [B,24,3]

    # FK chain
    Tw_R = np.empty((B, J, 3, 3))
    Tw_t = np.empty((B, J, 3))
    Tw_R[:, 0] = rot[:, 0]
    Tw_t[:, 0] = Jts[:, 0]
    for j in range(1, J):
        p_ = SMPL_PARENTS[j]
        rel = Jts[:, j] - Jts[:, p_]
        Tw_R[:, j] = Tw_R[:, p_] @ rot[:, j]
        Tw_t[:, j] = (Tw_R[:, p_] @ rel[..., None])[..., 0] + Tw_t[:, p_]
    A_t = Tw_t - np.einsum('bjmc,bjc->bjm', Tw_R, Jts)            # [B,24,3]
    return pf, Tw_R, A_t


def kernel(pose, betas, trans, v_template, shapedirs, posedirs, J_regressor,
           weights, parents):
    global LAST_RESULTS
    pose = np.asarray(pose, np.float32)
    betas = np.asarray(betas, np.float32)
    trans = np.asarray(trans, np.float32)
    v_template = np.asarray(v_template, np.float32)
    shapedirs = np.asarray(shapedirs, np.float32)
    posedirs = np.asarray(posedirs, np.float32)
    J_regressor = np.asarray(J_regressor, np.float32)
    weights = np.asarray(weights, np.float32)

    pf, Tw_R, A_t = _host_prep(pose, betas, trans, v_template, shapedirs,
                               J_regressor)

    # phi rows: [pf(207); betas(10); ones(1)]
    phi = np.concatenate([pf.T, betas.T, np.ones((1, B))], axis=0)
    phiA = phi[0:P].astype(ml_dtypes.bfloat16)
    phiB = phi[P:KD].astype(ml_dtypes.bfloat16)

    # TR4 [128, 3, B]
    tr4 = np.zeros((P, 3, B), np.float32)
    for q, slot in enumerate(SLOTS):
        b_, r_ = q // 4, q % 4
        if slot[0] == 't':
            m = slot[1]
            tr4[32 * r_:32 * r_ + J, b_] = A_t[:, :, m].T
            tr4[32 * r_ + J, b_] = trans[:, m]
        else:
            m, c = slot[1], slot[2]
            tr4[32 * r_:32 * r_ + J, b_] = Tw_R[:, :, m, c].T
    tr4 = tr4.reshape(P, 3 * B).astype(ml_dtypes.bfloat16)

    VTOT = VL * NCORES
    sd_p = np.zeros((VTOT, 3, NB), np.float32); sd_p[:V] = shapedirs
    vt_p = np.zeros((VTOT, 3), np.float32); vt_p[:V] = v_template
    w_p = np.zeros((VTOT, J), np.float32); w_p[:V] = weights
    pd_p = np.zeros((NPF, VTOT, 3), np.float32)
    pd_p[:, :V, :] = posedirs.reshape(NPF, V, 3)

    in_maps = []
    for core in range(NCORES):
        vsl = slice(core * VL, (core + 1) * VL)
        big = np.empty((KD, 3, VL), np.float32)
        big[0:NPF] = pd_p[:, vsl, :].transpose(0, 2, 1)   # [207, 3, VL]
        big[NPF:NPF + NB] = sd_p[vsl].transpose(2, 1, 0)  # [10, 3, VL]
        big[KD - 1] = vt_p[vsl].T                         # [3, VL]
        big = big.reshape(KD, 3 * VL)
        wtr4 = np.zeros((P, VL), np.float32)
        for r in range(4):
            wtr4[32 * r:32 * r + J] = w_p[vsl].T
            wtr4[32 * r + J] = 1.0
        in_maps.append({
            "bigA": np.ascontiguousarray(big[0:P]).astype(ml_dtypes.bfloat16),
            "bigB": np.ascontiguousarray(big[P:KD]).astype(ml_dtypes.bfloat16),
            "wtr4": wtr4.astype(ml_dtypes.bfloat16),
            "phiA": phiA,
            "phiB": phiB,
            "tr4": tr4,
        })

    nc = _get_nc()
    res = run_bass_kernel_spmd(nc, in_maps, core_ids=list(range(NCORES)))
    LAST_RESULTS = res

    verts = np.empty((B, V, 3), np.float32)
    for core in range(NCORES):
        lo = core * VL
        n = min(VL, V - lo)
        if n <= 0:
            break
        o = np.asarray(res.results[core]["out_v"]).astype(np.float32)
        o = o.reshape(VL, 3, B)
        verts[:, lo:lo + n, :] = o[:n].transpose(2, 0, 1)
    return verts


if __name__ == "__main__":
    rng = np.random.default_rng(0)
    ins = dict(
        pose=rng.standard_normal((B, J * 3)).astype(np.float32) * 0.2,
        betas=rng.standard_normal((B, NB)).astype(np.float32),
        trans=rng.standard_normal((B, 3)).astype(np.float32) * 0.1,
        v_template=rng.standard_normal((V, 3)).astype(np.float32) * 0.5,
        shapedirs=rng.standard_normal((V, 3, NB)).astype(np.float32) * 0.01,
        posedirs=rng.standard_normal((NPF, V * 3)).astype(np.float32) * 0.01,
        J_regressor=np.abs(rng.standard_normal((J, V)).astype(np.float32)),
        weights=np.abs(rng.standard_normal((V, J)).astype(np.float32)),
        parents=SMPL_PARENTS.copy(),
    )
    out = kernel(**ins)
    print("out", out.shape, out.dtype, np.abs(out).max())


# revision 33
# speedup vs baseline: 1.0383x; 1.0383x over previous
"""SMPL body-model (LBS) kernel for 8 Trainium2 NeuronCores.

Sharding: vertices split across the 8 cores (V=6890 -> 896/core padded);
batch (B=512) replicated on every core.  The host does the tiny per-batch
prep (Rodrigues rotations, the 24-joint forward-kinematics chain, input
layout); the cores run the heavy vertex math:

  vp[b,v,c]    = v_template + shapedirs@betas + posedirs@posefeat
                 (one K=218 matmul per vertex chunk)
  verts[b,v,m] = sum_j w[v,j]*A_t[b,j,m] + trans[b,m]
               + sum_c (sum_j w[v,j]*A_R[b,j,m,c]) * vp[b,v,c]

The 12 skinning contractions per 128-vertex chunk (9 S(m,c) K=24 and
3 At K=25) are packed into distinct 32-row PE tile positions; the
(v,b)-elementwise products and sums are spread across DVE / ACT / GpSimd.
Output is bf16, upcast on the host.
"""
import sys

sys.path.insert(0, '/opt/trn_rl_repo')

import contextlib

import ml_dtypes
import numpy as np

import concourse.bass as bass
import concourse.mybir as mybir
import concourse.tile as tile
from concourse import bacc
from concourse.bass_utils import run_bass_kernel_spmd
from concourse.masks import make_identity

P = 128
B = 512
J = 24
NB = 10
V = 6890
NCORES = 8
VL = 896             # vertices per core (padded)
VC = VL // P         # 7 vertex chunks per core
NPF = 207            # pose-feature length
KD = NPF + NB + 1    # 218 = vp contraction dim; rows [pf(207); betas(10); 1]
KB = KD - P          # 90 = second K chunk

SMPL_PARENTS = np.array([-1, 0, 0, 0, 1, 2, 3, 4, 5, 6, 7, 8, 9, 9, 9, 12,
                         13, 14, 16, 17, 18, 19, 20, 21], dtype=np.int32)

F32 = mybir.dt.float32
BF16 = mybir.dt.bfloat16
MUL = mybir.AluOpType.mult
ADD = mybir.AluOpType.add

# TR4 wave layout (column-batch b_, 32-row group r_):
#   W1 (b_=0): [At0, At1, At2, S22]   At rows get +trans at row 24
#   W2 (b_=1): [S00, S01, S02, S10]
#   W3 (b_=2): [S11, S12, S20, S21]
SLOTS = [('t', 0), ('t', 1), ('t', 2), ('s', 2, 2),
         ('s', 0, 0), ('s', 0, 1), ('s', 0, 2), ('s', 1, 0),
         ('s', 1, 1), ('s', 1, 2), ('s', 2, 0), ('s', 2, 1)]
# products in DRAIN_MC are ACT-drained to bf16 first (cheap DVE multiply);
# the rest multiply straight from PSUM on DVE.  GpSimd is kept out of the
# main loop (it contends with DVE for the shared SBUF port).
DRAIN_MC = ((0, 0), (0, 1), (1, 0), (1, 1), (2, 0))

LAST_RESULTS = None  # for the local test harness


def build_kernel():
    nc = bacc.Bacc("TRN2", target_bir_lowering=False, debug=False,
                   num_devices=NCORES)

    d_bigA = nc.dram_tensor("bigA", [P, 3 * VL], BF16, kind="ExternalInput")
    d_bigB = nc.dram_tensor("bigB", [KB, 3 * VL], BF16, kind="ExternalInput")
    d_wtr4 = nc.dram_tensor("wtr4", [P, VL], BF16, kind="ExternalInput")
    d_phiA = nc.dram_tensor("phiA", [P, B], BF16, kind="ExternalInput")
    d_phiB = nc.dram_tensor("phiB", [KB, B], BF16, kind="ExternalInput")
    d_tr4 = nc.dram_tensor("tr4", [P, 3 * B], BF16, kind="ExternalInput")
    d_out = nc.dram_tensor("out_v", [VL, 3 * B], BF16, kind="ExternalOutput")

    with tile.TileContext(nc) as tc, contextlib.ExitStack() as ctx:
        singles = ctx.enter_context(tc.tile_pool(name="singles", bufs=1))

        phiA = singles.tile([P, B], BF16)
        nc.sync.dma_start(phiA[:], d_phiA[:, :])
        phiB = singles.tile([KB, B], BF16)
        nc.sync.dma_start(phiB[:], d_phiB[:, :])
        TR4 = singles.tile([P, 3, B], BF16)
        nc.sync.dma_start(TR4[:], d_tr4[:, :].rearrange("p (t b) -> p t b", t=3))
        wtr4_sb = singles.tile([P, VL], BF16)
        nc.sync.dma_start(wtr4_sb[:], d_wtr4[:, :])
        # column-chunked loads: the first chunk of each c-region lands first
        # so the vc=0/1 blendshape matmuls can start early
        bigA_sb = singles.tile([P, 3 * VL], BF16)
        bigB_sb = singles.tile([KB, 3 * VL], BF16)
        for half in range(3):
            lo = [0, 2 * P, VL][half]
            hi = [2 * P, VL, VL][half]
            if lo == hi:
                continue
            for c in range(3):
                nc.sync.dma_start(bigA_sb[:, c * VL + lo:c * VL + hi],
                                  d_bigA[:, c * VL + lo:c * VL + hi])
                nc.sync.dma_start(bigB_sb[:, c * VL + lo:c * VL + hi],
                                  d_bigB[:, c * VL + lo:c * VL + hi])
        ident_bf = singles.tile([P, P], BF16)
        make_identity(nc, ident_bf[:])

        dvp_pool = ctx.enter_context(tc.tile_pool(name="dvp", bufs=3))
        tmp_pool = ctx.enter_context(tc.tile_pool(name="tmpmc", bufs=2))
        out_pool = ctx.enter_context(tc.tile_pool(name="outs", bufs=2))
        ps_d = ctx.enter_context(tc.tile_pool(name="psD", bufs=2, space="PSUM"))
        ps_s = ctx.enter_context(tc.tile_pool(name="psS", bufs=2, space="PSUM"))
        ps_t = ctx.enter_context(tc.tile_pool(name="psT", bufs=1, space="PSUM"))

        def emit_dvp(vc):
            dvp_sb = dvp_pool.tile([P, 3, B], BF16, tag="dvp")
            for c in range(3):
                dps = ps_d.tile([P, B], F32, tag="dvpp")
                nc.tensor.matmul(
                    dps[:],
                    bigA_sb[:, c * VL + vc * P: c * VL + (vc + 1) * P],
                    phiA[:], start=True, stop=False)
                nc.tensor.matmul(
                    dps[:],
                    bigB_sb[:, c * VL + vc * P: c * VL + (vc + 1) * P],
                    phiB[:], start=False, stop=True)
                nc.scalar.copy(dvp_sb[:, c, :], dps[:])
            return dvp_sb

        # PE warm-up: a gapless burst during the input-DMA head flips the
        # HAM clock gate to 8/8 (2.4 GHz) before the real matmul stream.
        warm = ps_d.tile([P, B], F32, tag="dvpp")
        for _ in range(36):
            nc.tensor.matmul(warm[:, 0:P], ident_bf[:], ident_bf[:],
                             start=True, stop=True)

        dvp_tiles = {}
        dvp_tiles[0] = emit_dvp(0)
        dvp_tiles[1] = emit_dvp(1)

        # short bridge burst: splits the PE-idle wait for the skinning
        # operands (TR4/wtr4) so no idle stretch reaches the ~3.4us HAM
        # re-throttle window
        warm2 = ps_d.tile([P, B], F32, tag="dvpp")
        for _ in range(12):
            nc.tensor.matmul(warm2[:, 0:P], ident_bf[:], ident_bf[:],
                             start=True, stop=True)

        for vc in range(VC):
            vsl = slice(vc * P, (vc + 1) * P)
            dvp_sb = dvp_tiles.pop(vc)
            s_ps = {}
            t_ps = {}
            for q, slot in enumerate(SLOTS):
                b_, r_ = q // 4, q % 4
                if slot[0] == 't':
                    m = slot[1]
                    tp = ps_t.tile([P, B], F32, tag=f"tps{m}", bufs=1)
                    nc.tensor.matmul(tp[:],
                                     wtr4_sb[32 * r_:32 * r_ + J + 1, vsl],
                                     TR4[32 * r_:32 * r_ + J + 1, b_, :],
                                     start=True, stop=True,
                                     tile_position=(32 * r_, 0))
                    t_ps[m] = tp
                else:
                    m, c = slot[1], slot[2]
                    if (m, c) == (2, 2):
                        sp = ps_s.tile([P, B], F32, tag="s22", bufs=1)
                    else:
                        sp = ps_s.tile([P, B], F32, tag="sps")
                    nc.tensor.matmul(sp[:], wtr4_sb[32 * r_:32 * r_ + J, vsl],
                                     TR4[32 * r_:32 * r_ + J, b_, :],
                                     start=True, stop=True,
                                     tile_position=(32 * r_, 0))
                    s_ps[(m, c)] = sp
            # prefetch the vp tile two chunks ahead
            if vc + 2 < VC:
                dvp_tiles[vc + 2] = emit_dvp(vc + 2)
            # drain the At banks (needed in SBUF for the final merges)
            tt_sb = {}
            for m in range(3):
                tb = tmp_pool.tile([P, B], BF16, tag=f"tt{m}")
                nc.scalar.copy(tb[:], t_ps[m][:])
                tt_sb[m] = tb
            # products P(m,c) = S(m,c) * vp[c], straight from PSUM on DVE
            prod = {}
            for mc in [(0, 0), (0, 1), (0, 2), (1, 0), (1, 1), (1, 2),
                       (2, 0), (2, 1), (2, 2)]:
                m, c = mc
                pm = tmp_pool.tile([P, B], BF16, tag=f"pr{m}{c}")
                if mc in DRAIN_MC:
                    sb = tmp_pool.tile([P, B], BF16, tag=f"sdr{m}{c}")
                    nc.scalar.copy(sb[:], s_ps[mc][:])
                    nc.vector.tensor_tensor(pm[:], sb[:],
                                            dvp_sb[:, c, :], MUL)
                else:
                    nc.vector.tensor_tensor(pm[:], s_ps[mc][:],
                                            dvp_sb[:, c, :], MUL)
                prod[mc] = pm
            # shallow add tree per m: (P0+P1) + (P2+At) on DVE
            for m in range(3):
                t01 = tmp_pool.tile([P, B], BF16, tag=f"t01_{m}")
                nc.vector.tensor_tensor(t01[:], prod[(m, 0)][:],
                                        prod[(m, 1)][:], ADD)
                t2t = tmp_pool.tile([P, B], BF16, tag=f"t2t_{m}")
                nc.vector.tensor_tensor(t2t[:], prod[(m, 2)][:],
                                        tt_sb[m][:], ADD)
                vo = tmp_pool.tile([P, B], BF16, tag=f"vo_{m}")
                nc.vector.tensor_tensor(vo[:], t01[:], t2t[:], ADD)
                nc.sync.dma_start(d_out[vsl, m * B:(m + 1) * B], vo[:])

    nc.compile()
    return nc


_NC_CACHE = None


def _get_nc():
    global _NC_CACHE
    if _NC_CACHE is None:
        _NC_CACHE = build_kernel()
    return _NC_CACHE


def _host_prep(pose, betas, trans, v_template, shapedirs, J_regressor):
    """Per-batch prep: Rodrigues + FK chain -> A matrices, pose features."""
    rvec = pose.reshape(B, J, 3).astype(np.float64)
    angle = np.sqrt((rvec * rvec).sum(-1, keepdims=True) + 1e-16)
    axis = rvec / angle
    s = np.sin(angle)[..., None]
    c = np.cos(angle)[..., None]
    x, y, z = axis[..., 0], axis[..., 1], axis[..., 2]
    zero = np.zeros_like(x)
    K = np.stack([zero, -z, y, z, zero, -x, -y, x, zero],
                 axis=-1).reshape(B, J, 3, 3)
    outer = axis[..., :, None] * axis[..., None, :]
    rot = c * np.eye(3)[None, None] + (1.0 - c) * outer + s * K  # [B,J,3,3]

    pf = (rot[:, 1:] - np.eye(3)[None, None]).reshape(B, NPF)    # [B,207]

    # joint rest positions from shape blendshapes
    j0 = J_regressor @ v_template                                 # [24,3]
    jd = (J_regressor @ shapedirs.reshape(V, 30)).reshape(J, 3, NB)
    Jts = j0[None] + np.einsum('bk,jck->bjc', betas.astype(np.float64),
                               jd.astype(np.float64))             # [B,24,3]

    # FK chain
    Tw_R = np.empty((B, J, 3, 3))
    Tw_t = np.empty((B, J, 3))
    Tw_R[:, 0] = rot[:, 0]
    Tw_t[:, 0] = Jts[:, 0]
    for j in range(1, J):
        p_ = SMPL_PARENTS[j]
        rel = Jts[:, j] - Jts[:, p_]
        Tw_R[:, j] = Tw_R[:, p_] @ rot[:, j]
        Tw_t[:, j] = (Tw_R[:, p_] @ rel[..., None])[..., 0] + Tw_t[:, p_]
    A_t = Tw_t - np.einsum('bjmc,bjc->bjm', Tw_R, Jts)            # [B,24,3]
    return pf, Tw_R, A_t


def kernel(pose, betas, trans, v_template, shapedirs, posedirs, J_regressor,
           weights, parents):
    global LAST_RESULTS
    pose = np.asarray(pose, np.float32)
    betas = np.asarray(betas, np.float32)
    trans = np.asarray(trans, np.float32)
    v_template = np.asarray(v_template, np.float32)
    shapedirs = np.asarray(shapedirs, np.float32)
    posedirs = np.asarray(posedirs, np.float32)
    J_regressor = np.asarray(J_regressor, np.float32)
    weights = np.asarray(weights, np.float32)

    pf, Tw_R, A_t = _host_prep(pose, betas, trans, v_template, shapedirs,
                               J_regressor)

    # phi rows: [pf(207); betas(10); ones(1)]
    phi = np.concatenate([pf.T, betas.T, np.ones((1, B))], axis=0)
    phiA = phi[0:P].astype(ml_dtypes.bfloat16)
    phiB = phi[P:KD].astype(ml_dtypes.bfloat16)

    # TR4 [128, 3, B]
    tr4 = np.zeros((P, 3, B), np.float32)
    for q, slot in enumerate(SLOTS):
        b_, r_ = q // 4, q % 4
        if slot[0] == 't':
            m = slot[1]
            tr4[32 * r_:32 * r_ + J, b_] = A_t[:, :, m].T
            tr4[32 * r_ + J, b_] = trans[:, m]
        else:
            m, c = slot[1], slot[2]
            tr4[32 * r_:32 * r_ + J, b_] = Tw_R[:, :, m, c].T
    tr4 = tr4.reshape(P, 3 * B).astype(ml_dtypes.bfloat16)

    VTOT = VL * NCORES
    sd_p = np.zeros((VTOT, 3, NB), np.float32); sd_p[:V] = shapedirs
    vt_p = np.zeros((VTOT, 3), np.float32); vt_p[:V] = v_template
    w_p = np.zeros((VTOT, J), np.float32); w_p[:V] = weights
    pd_p = np.zeros((NPF, VTOT, 3), np.float32)
    pd_p[:, :V, :] = posedirs.reshape(NPF, V, 3)

    in_maps = []
    for core in range(NCORES):
        vsl = slice(core * VL, (core + 1) * VL)
        big = np.empty((KD, 3, VL), np.float32)
        big[0:NPF] = pd_p[:, vsl, :].transpose(0, 2, 1)   # [207, 3, VL]
        big[NPF:NPF + NB] = sd_p[vsl].transpose(2, 1, 0)  # [10, 3, VL]
        big[KD - 1] = vt_p[vsl].T                         # [3, VL]
        big = big.reshape(KD, 3 * VL)
        wtr4 = np.zeros((P, VL), np.float32)
        for r in range(4):
            wtr4[32 * r:32 * r + J] = w_p[vsl].T
            wtr4[32 * r + J] = 1.0
        in_maps.append({
            "bigA": np.ascontiguousarray(big[0:P]).astype(ml_dtypes.bfloat16),
            "bigB": np.ascontiguousarray(big[P:KD]).astype(ml_dtypes.bfloat16),
            "wtr4": wtr4.astype(ml_dtypes.bfloat16),
            "phiA": phiA,
            "phiB": phiB,
            "tr4": tr4,
        })

    nc = _get_nc()
    res = run_bass_kernel_spmd(nc, in_maps, core_ids=list(range(NCORES)))
    LAST_RESULTS = res

    verts = np.empty((B, V, 3), np.float32)
    for core in range(NCORES):
        lo = core * VL
        n = min(VL, V - lo)
        if n <= 0:
            break
        o = np.asarray(res.results[core]["out_v"]).astype(np.float32)
        o = o.reshape(VL, 3, B)
        verts[:, lo:lo + n, :] = o[:n].transpose(2, 0, 1)
    return verts


if __name__ == "__main__":
    rng = np.random.default_rng(0)
    ins = dict(
        pose=rng.standard_normal((B, J * 3)).astype(np.float32) * 0.2,
        betas=rng.standard_normal((B, NB)).astype(np.float32),
        trans=rng.standard_normal((B, 3)).astype(np.float32) * 0.1,
        v_template=rng.standard_normal((V, 3)).astype(np.float32) * 0.5,
        shapedirs=rng.standard_normal((V, 3, NB)).astype(np.float32) * 0.01,
        posedirs=rng.standard_normal((NPF, V * 3)).astype(np.float32) * 0.01,
        J_regressor=np.abs(rng.standard_normal((J, V)).astype(np.float32)),
        weights=np.abs(rng.standard_normal((V, J)).astype(np.float32)),
        parents=SMPL_PARENTS.copy(),
    )
    out = kernel(**ins)
    print("out", out.shape, out.dtype, np.abs(out).max())
